# revision 35
# baseline (speedup 1.0000x reference)
"""Multi-head attention forward (B=2, S=2048, D=1024, H=16) on 8 Trainium2
NeuronCores, tensor-parallel over heads (2 heads per core).

fp16 datapath with an fp8e4m3 DoubleRow context stage, software-pipelined:
  - All projection/score matmul operands fp16 (1 col/cycle PE stream rate);
    PSUM accumulation fp32.
  - bk is dropped entirely (a per-q constant shift cancels in softmax);
    bv is folded into the host-side constant (Wo @ bv + bo); only bq is
    added on-chip (fused into the q-projection PSUM drain).
  - scoresT[k, q] = kT_h.T @ qT_h per (batch, head); exp via ScalarE with
    1/sqrt(64) folded into the activation scale.
  - ctx: k-tiles 0..11 of each batch run as fp8e4m3 DoubleRow pair-matmuls
    (contraction 256/pass, halving PE passes): stationary = (aug_kt,
    aug_kt+1) fp8 tiles adjacent in SBUF, moving = the exp'd probs written
    as fp8 [128, 2, 512] (adjacent tiles, the natural activation layout).
    k-tiles 12..15 stay fp16 (probs fp16 + fp16 aug) to keep the fp8
    quantization error inside the harness tolerance. The ones column in
    aug gives the softmax denominators for free in both paths.
  - v transposed into aug16 via the DMA XBAR (fp16), then one DVE cast per
    chunk produces the fp8 aug tiles.
  - normalize: broadcast the sums row with a PE ones-matmul,
    reciprocal_approx_fast on DVE, multiply into fp16 ctxT; output
    projection per 128-token tile, fp16 partials DMA'd out; host sums the
    8 partials and adds (Wo @ bv + bo) in fp32.
  - schedule: attention(0) starts after one projected chunk; remaining
    projections and out_proj tiles are emitted as fine-grained fillers
    inside the attention kp loops, so the PE never idles long enough for
    the HAM clock gate to re-throttle and exp-wait stalls are absorbed.
    The tail runs as two parallel copy+DMA chains on ScalarE/DVE.

Predecessor (all-fp16): 233.4us.
"""
import sys
import os

sys.path.insert(0, '/opt/trn_rl_repo')

import numpy as np
import ml_dtypes
import concourse.bass as bass
import concourse.mybir as mybir
import concourse.tile as tile
from concourse import bacc, bass_utils
import contextlib

f32 = mybir.dt.float32
f16 = mybir.dt.float16
f8 = mybir.dt.float8e4
np8 = ml_dtypes.float8_e4m3
EXP = mybir.ActivationFunctionType.Exp
DR = mybir.MatmulPerfMode.DoubleRow

B, S, D, H, HD = 2, 2048, 1024, 16, 64
T = B * S              # 4096 tokens
DC = 128               # dims per core (2 heads)
KT = 8                 # feature k-tiles (D / 128)
NCH = 8                # projection chunks of 512 tokens
NKT = 16               # k-token tiles per batch (S / 128)
NQC = 4                # q chunks of 512 per (b, h)
NKP8 = 8               # kp-pairs (of 8) on the fp8 DoubleRow ctx path


def _build():
    nc = bacc.Bacc("TRN2", target_bir_lowering=False, debug=False)
    xT_d = nc.dram_tensor("xT", [D, T], f16, kind="ExternalInput").ap()
    wqT_d = nc.dram_tensor("wqT", [D, DC], f16, kind="ExternalInput").ap()
    wkT_d = nc.dram_tensor("wkT", [D, DC], f16, kind="ExternalInput").ap()
    wvT_d = nc.dram_tensor("wvT", [D, DC], f16, kind="ExternalInput").ap()
    woT_d = nc.dram_tensor("woT", [DC, D], f16, kind="ExternalInput").ap()
    bq_d = nc.dram_tensor("bq", [DC, 1], f32, kind="ExternalInput").ap()
    out_d = nc.dram_tensor("out", [T, D], f16, kind="ExternalOutput").ap()

    xT_ap = xT_d.rearrange("(kt p) t -> p kt t", p=128)

    with tile.TileContext(nc) as tc:
        ctx = contextlib.ExitStack()
        cpool = ctx.enter_context(tc.tile_pool(name="cpool", bufs=1))
        xpool = ctx.enter_context(tc.tile_pool(name="xpool", bufs=4))
        ppool = ctx.enter_context(tc.tile_pool(name="ppool", bufs=8))
        npool = ctx.enter_context(tc.tile_pool(name="npool", bufs=3))
        opool = ctx.enter_context(tc.tile_pool(name="opool", bufs=4))
        pj = ctx.enter_context(tc.tile_pool(name="pj", bufs=2, space="PSUM"))
        sc = ctx.enter_context(tc.tile_pool(name="sc", bufs=2, space="PSUM"))
        cx = ctx.enter_context(tc.tile_pool(name="cx", bufs=2, space="PSUM"))

        # ---- constants / persistent tiles ----
        wqr = cpool.tile([128, KT, DC], f16, tag="wqr")
        wkr = cpool.tile([128, KT, DC], f16, tag="wkr")
        wvr = cpool.tile([128, KT, DC], f16, tag="wvr")
        # k-weights first ON THE HWDGE SYNC QUEUE: the k0 projection is the
        # first PE consumer and the gpsimd SWDGE queue starts ~1us late
        nc.sync.dma_start(wkr[:], wkT_d.rearrange("(kt p) m -> p kt m", p=128))
        nc.gpsimd.dma_start(wqr[:], wqT_d.rearrange("(kt p) m -> p kt m", p=128))
        nc.gpsimd.dma_start(wvr[:], wvT_d.rearrange("(kt p) m -> p kt m", p=128))
        wor = cpool.tile([128, D], f16, tag="wor")
        nc.gpsimd.dma_start(wor[:], woT_d[:])
        bq = cpool.tile([DC, 1], f32, tag="bq")
        nc.gpsimd.dma_start(bq[:], bq_d[:])

        ones = cpool.tile([128, 128], f16, tag="ones")
        nc.vector.memset(ones[:], 1.0)

        # aug16[p, kt, h, :]: fp16 augmented V operand (XBAR target + the
        # fp16 ctx stationary for k-tiles 12..15).
        # h0: v dims at cols 0..63, ones col 64 -> ctx rows 0..63, sums 64
        # h1: v dims at cols 64..127, ones col 0 -> ctx rows 64..127, sums 0
        aug16 = cpool.tile([128, B * NKT, 2, 128], f16, tag="aug16")
        nc.vector.memset(aug16[:], 0.0)
        nc.vector.tensor_copy(aug16[:, :, 0, 64:65], ones[:, 0:B * NKT])
        nc.vector.tensor_copy(aug16[:, :, 1, 0:1], ones[:, 0:B * NKT])

        # aug8[p, b, kp, h, j, :]: fp8 cast of aug16 for the DoubleRow ctx
        # path, the two k-tiles of a pair adjacent (dual-fp8 ldweights
        # requires tile stride == tile width)
        aug8 = cpool.tile([128, B, NKP8, 2, 2, 128], f8, tag="aug8")

        qTr = cpool.tile([128, T], f16, tag="qTr")
        kTr = cpool.tile([128, T], f16, tag="kTr")
        vTs = cpool.tile([128, T], f16, tag="vTs")
        ctxT = [cpool.tile([128, S], f16, tag=f"ctxT{b}", name=f"ctxT{b}")
                for b in range(B)]

        # preload the Exp activation table off the critical path
        warmact = cpool.tile([1, 1], f32, tag="warmact")
        nc.scalar.activation(warmact[0:1, 0:1], ctxT[1][0:1, 0:1], EXP)

        # ---- phase 1: projections + v transposes ----
        _xtiles = {}

        def load_chunk(ch):
            csl = slice(ch * 512, (ch + 1) * 512)
            xTr = xpool.tile([128, KT, 512], f16, tag="xTr")
            if ch == 0:
                # separate tiles per feature block: tile-granular dependency
                # tracking lets the first projection matmul start as soon as
                # f=0 lands instead of waiting for the whole chunk
                xts = []
                for f in range(KT):
                    xf = cpool.tile([128, 512], f16, tag=f"x0f{f}",
                                    name=f"x0f{f}")
                    nc.sync.dma_start(xf[:], xT_ap[:, f, csl])
                    xts.append(xf)
                _xtiles[0] = xts
                return
            else:
                # batch-1 chunks go on the gpsimd SWDGE queue: the SP queue
                # is saturated with XBAR transposes during attention(0)
                eng = nc.sync if ch < 4 else nc.gpsimd
                eng.dma_start(xTr[:], xT_ap[:, :, csl])
            _xtiles[ch] = xTr

        def proj_one(ch, wr, b_t, dst):
            csl = slice(ch * 512, (ch + 1) * 512)
            xTr = _xtiles[ch]
            pp = pj.tile([128, 512], f32, tag="pj")
            for f in range(KT):
                xf = xTr[f][:] if ch == 0 else xTr[:, f]
                nc.tensor.matmul(pp[:], wr[:, f], xf,
                                 start=(f == 0), stop=(f == KT - 1))
            if b_t is None:
                nc.vector.tensor_copy(dst[:, csl], pp[:])
            else:
                nc.vector.tensor_scalar_add(dst[:, csl], pp[:], b_t[:])

        def transp_chunk(ch):
            # transpose v into aug16 via the DMA XBAR (one transfer per head
            # covers this chunk's 4 k-token tiles), then cast the fp8 pairs
            csl = slice(ch * 512, (ch + 1) * 512)
            kts = slice(ch * 4, (ch + 1) * 4)
            for h in range(2):
                nc.sync.dma_start(
                    aug16[:, kts, h, h * 64:(h + 1) * 64],
                    vTs[h * 64:(h + 1) * 64, csl], transpose=True)
            b, ch4 = divmod(ch, 4)
            nkp = 2 if ch4 < 3 else NKP8 - 6  # chunk 3: only kt 12..13
            if nkp > 0:
                kts8 = slice(ch * 4, ch * 4 + 2 * nkp)
                src = aug16[:, kts8, :, :].rearrange(
                    "p (kp j) h c -> p kp j h c", kp=nkp)
                dst = aug8[:, b, ch4 * 2:ch4 * 2 + nkp, :, :, :].rearrange(
                    "p kp h j c -> p kp j h c")
                nc.vector.tensor_copy(dst, src)

        # ---- phase 2/3: attention + output projection ----
        def attention_qc(b, qc, fillers=None, split_norm=False):
            qsl = slice(b * S + qc * 512, b * S + (qc + 1) * 512)
            osl = slice(qc * 512, (qc + 1) * 512)
            for h in range(2):
                hs = slice(h * 64, (h + 1) * 64)
                vrows = slice(0, 64) if h == 0 else slice(64, 128)
                srow = 64 if h == 0 else 0  # psum row holding the exp sums
                ctxp = cx.tile([128, 512], f32, tag="cx", name="ctxp")
                # two kt's scoresT share one 2-bank psum tile so a single
                # 1024-wide exp covers both.  ctx(kp) is emitted AFTER
                # scores(kp+1) so the PE streams scores during exp(kp).
                pend_ctx = None
                for kp in range(NKT // 2):
                    if fillers and (h, kp) in fillers:
                        for fn in fillers[(h, kp)]:
                            fn()
                    fp8_kp = kp < NKP8
                    scp = sc.tile([128, 1024], f32, tag="sc", name="scp")
                    for j in range(2):
                        kt = kp * 2 + j
                        ksl = slice((b * NKT + kt) * 128,
                                    (b * NKT + kt + 1) * 128)
                        nc.tensor.matmul(scp[:, j * 512:(j + 1) * 512],
                                         kTr[hs, ksl], qTr[hs, qsl],
                                         start=True, stop=True)
                    if fp8_kp:
                        probs = ppool.tile([128, 2, 512], f8, tag="pb8",
                                           name="probs")
                        nc.scalar.activation(
                            probs[:].rearrange("p a c -> p (a c)"), scp[:],
                            EXP, scale=0.125)
                    else:
                        probs = ppool.tile([128, 1024], f16, tag="pb16",
                                           name="probs")
                        nc.scalar.activation(probs[:], scp[:], EXP,
                                             scale=0.125)
                    if pend_ctx is not None:
                        pend_ctx()

                    def pend_ctx(kp=kp, probs=probs, fp8_kp=fp8_kp):
                        if fp8_kp:
                            nc.tensor.matmul(
                                ctxp[:], aug8[:, b, kp, h, :, :], probs[:],
                                start=(kp == 0), stop=(kp == NKT // 2 - 1),
                                perf_mode=DR)
                        else:
                            for j in range(2):
                                kt = kp * 2 + j
                                nc.tensor.matmul(
                                    ctxp[:], aug16[:, b * NKT + kt, h, :],
                                    probs[:, j * 512:(j + 1) * 512],
                                    start=False, stop=(kt == NKT - 1))
                pend_ctx()
                # broadcast the sums row across the ctx partitions via a
                # PE ones-matmul, reciprocal on DVE, then normalize.
                srt = npool.tile([128, 512], f16, tag="srt")
                nc.vector.tensor_copy(srt[srow:srow + 1, :],
                                      ctxp[srow:srow + 1, :])
                bcp = cx.tile([128, 512], f32, tag="cx", name="bcp")
                nc.tensor.matmul(bcp[:, :], ones[srow:srow + 1, :],
                                 srt[srow:srow + 1, :], start=True, stop=True)
                bcs = npool.tile([128, 512], f32, tag="bcs")
                nc.vector.reciprocal_approx_fast(bcs[:, :], bcp[:, :])
                if split_norm and h == 1:
                    # final group: write ctxT in 128-token slices so the
                    # tail out-proj tiles can start before the whole qc is
                    # normalized
                    for p4 in range(4):
                        psl = slice(p4 * 128, (p4 + 1) * 128)
                        olo = qc * 512 + p4 * 128
                        nc.vector.tensor_mul(ctxT[b][vrows, olo:olo + 128],
                                             ctxp[vrows, psl],
                                             bcs[vrows, psl])
                else:
                    nc.vector.tensor_mul(ctxT[b][vrows, osl], ctxp[vrows, :],
                                         bcs[vrows, :])

        def out_proj_tiles(b, tts, copy_eng=None, dma_eng=None):
            for tt in tts:
                ost = opool.tile([128, D], f16, tag="ost", name="ost")
                for oc in range(2):
                    op = pj.tile([128, 512], f32, tag="pj", name="op")
                    nc.tensor.matmul(op[:], ctxT[b][:, tt * 128:(tt + 1) * 128],
                                     wor[:, oc * 512:(oc + 1) * 512],
                                     start=True, stop=True)
                    if copy_eng is None:
                        nc.vector.tensor_copy(ost[:, oc * 512:(oc + 1) * 512],
                                              op[:])
                    else:
                        copy_eng(ost[:, oc * 512:(oc + 1) * 512], op[:])
                (dma_eng or nc.gpsimd).dma_start(
                    out_d[b * S + tt * 128:b * S + (tt + 1) * 128, :], ost[:])

        def scalar_copy(dst, src):
            nc.scalar.activation(dst, src, mybir.ActivationFunctionType.Copy)

        def P(ch, wr, b_t, dst):
            return lambda: proj_one(ch, wr, b_t, dst)

        def V(ch):
            def f():
                proj_one(ch, wvr, None, vTs)
                transp_chunk(ch)
            return f

        def OP(b, t0, t1):
            return lambda: out_proj_tiles(b, range(t0, t1))

        # Emission order = scheduler priority hint. Projections are emitted
        # as fillers inside the attention kp loops: scores for kp-pair j only
        # need the k/q/v chunks emitted before it, so attention(0) starts
        # after a single chunk and the batch transition self-paces.
        for ch in range(4):
            load_chunk(ch)
        for ch in range(4, NCH):
            load_chunk(ch)
        proj_one(0, wkr, None, kTr)
        proj_one(0, wqr, bq, qTr)
        proj_one(0, wvr, None, vTs)
        transp_chunk(0)
        proj_one(1, wkr, None, kTr)
        proj_one(1, wvr, None, vTs)
        transp_chunk(1)
        attention_qc(0, 0, fillers={
            (0, 0): [P(2, wkr, None, kTr), V(2)],
            (0, 2): [P(3, wkr, None, kTr)],
            (0, 4): [V(3)],
            (1, 0): [P(1, wqr, bq, qTr)]})
        attention_qc(0, 1, fillers={
            (0, 0): [P(2, wqr, bq, qTr)],
            (0, 4): [P(4, wkr, None, kTr)],
            (1, 0): [P(3, wqr, bq, qTr)],
            (1, 4): [OP(0, 0, 2)]})
        attention_qc(0, 2, fillers={
            (0, 0): [P(4, wqr, bq, qTr)],
            (0, 4): [OP(0, 2, 4)],
            (1, 0): [P(5, wkr, None, kTr)],
            (1, 4): [OP(0, 4, 6)]})
        attention_qc(0, 3, fillers={
            (0, 0): [P(5, wqr, bq, qTr)],
            (0, 4): [P(7, wkr, None, kTr)],
            (1, 0): [P(6, wkr, None, kTr)],
            (1, 4): [OP(0, 6, 8)]})
        attention_qc(1, 0, fillers={
            (0, 0): [V(4)], (0, 2): [V(5)],
            (0, 4): [V(6)], (0, 6): [V(7)],
            (1, 0): [P(6, wqr, bq, qTr)],
            (1, 2): [OP(0, 8, 10)], (1, 4): [OP(0, 10, 12)],
            (1, 6): [OP(0, 12, 14)]})
        attention_qc(1, 1, fillers={
            (0, 0): [OP(0, 14, 16)],
            (0, 4): [OP(1, 0, 2)],
            (1, 0): [P(7, wqr, bq, qTr)],
            (1, 4): [OP(1, 2, 4)]})
        # qc order 0,1,3,2 for batch 1: qc3's out-proj tiles (12-15) then
        # interleave into the final group, leaving only qc2's tiles (8-11)
        # for the tail (started early via the split normalize).
        attention_qc(1, 3, fillers={
            (0, 0): [OP(1, 4, 6)], (0, 4): [OP(1, 6, 7)],
            (1, 0): [OP(1, 7, 8)]})
        attention_qc(1, 2, split_norm=True, fillers={
            (0, 2): [OP(1, 12, 13)], (0, 4): [OP(1, 13, 14)],
            (0, 6): [OP(1, 14, 15)], (1, 0): [OP(1, 15, 16)]})
        # ScalarE is done with exp by now - run the tail as two independent
        # copy+DMA chains (ScalarE->Act HWDGE and DVE->SP HWDGE) in parallel
        out_proj_tiles(1, range(8, 9), copy_eng=scalar_copy,
                       dma_eng=nc.scalar)
        out_proj_tiles(1, range(9, 10), dma_eng=nc.sync)
        out_proj_tiles(1, range(10, 11), copy_eng=scalar_copy,
                       dma_eng=nc.scalar)
        out_proj_tiles(1, range(11, 12), dma_eng=nc.sync)
        ctx.close()

    nc.compile()
    return nc


_NC = None


def _in_maps(inputs, Wq, bq, Wk, bk, Wv, bv, Wo, bo):
    x = np.ascontiguousarray(np.asarray(inputs, dtype=np.float32).reshape(T, D))
    xT = np.ascontiguousarray(x.T.astype(np.float16))
    Wq = np.asarray(Wq, dtype=np.float32)
    Wk = np.asarray(Wk, dtype=np.float32)
    Wv = np.asarray(Wv, dtype=np.float32)
    Wo = np.asarray(Wo, dtype=np.float32)

    in_maps = []
    for c in range(8):
        sl = slice(c * DC, (c + 1) * DC)
        in_maps.append({
            "xT": xT,
            "wqT": np.ascontiguousarray(Wq[sl].T.astype(np.float16)),
            "wkT": np.ascontiguousarray(Wk[sl].T.astype(np.float16)),
            "wvT": np.ascontiguousarray(Wv[sl].T.astype(np.float16)),
            "woT": np.ascontiguousarray(Wo[:, sl].T.astype(np.float16)),
            "bq": np.ascontiguousarray(np.asarray(bq, np.float32)[sl][:, None]),
        })
    return in_maps


def kernel(inputs, Wq, bq, Wk, bk, Wv, bv, Wo, bo):
    global _NC
    if _NC is None:
        _NC = _build()

    in_maps = _in_maps(inputs, Wq, bq, Wk, bk, Wv, bv, Wo, bo)
    res = bass_utils.run_bass_kernel_spmd(_NC, in_maps, core_ids=list(range(8)))
    out = res.results[0]["out"].astype(np.float32)
    for r in res.results[1:]:
        out += r["out"].astype(np.float32)
    out += (np.asarray(Wo, np.float32) @ np.asarray(bv, np.float32)
            + np.asarray(bo, np.float32))[None, :]
    return out.reshape(B, S, D)
